# revision 28
# baseline (speedup 1.0000x reference)
"""Multi-head causal attention (B=4, T=2048, D=1024, H=16) on 8 Trainium2 cores.

Sharding: core c = (b, g) with b = c//2 (batch), g = c%2 (head-group of 8 heads).
Each core: Q/K/V projections for its 8 heads (column-parallel), causal attention,
row-parallel partial output projection. Host sums the g=0/g=1 partials + bias.

v4 design (fp8 DoubleRow + streaming AV; cost model: matmul = out-free-rows x
0.4167ns x cpr, fp8e4 DoubleRow cpr=0.5 contracting 2x128/instr; ACT exp =
0.8333ns/free-elem is the ~146us floor):
  - Q/K proj: fp8 DR, x8 moving [128,2,512], w8 stationary [128,2,128] in
    4 col-groups (t=pair-half, h=dim-half) so psum partitions land as
    (beta=2*(pr%2)+u, dd) blocks of 32 -> qT8/kT8 [32-blocks, 2h, T] fp8,
    ready for 2x32-contraction DR score matmuls. w quantized x16 (subnormal
    dodge), unscaled in the DVE psum->fp8 copy; score scale 1/8 folded as
    qT8 = e4m3(q/2) + exp(scale=0.25).
  - scores: fp8 DR per (pr,u,kj): lhsT = kT8 32x2x128, rhs = qT8, out ss
    [128 keys, u, 512-c0] psum; diag mask = DVE add of -1e9 maskb pre-exp.
  - exp: ACT psum->sbuf fp8 pt pairs [128, 2kj, 2u, 512]; diag garbage cols
    zeroed by Pool memset (bitwise, fp8-safe).
  - AV: streaming fp8 DR per (pr,u,kjp,hi/lo): lhsT = Vb [128,2kj,65]
    (64 v-dims + ones col -> Z at psum row 64), rhs = pt pair, out ctx
    [65, 512] psum accumulated across the span. V stored hi+lo e4m3 pair
    (residual split ~ fp16 quality at DR cost).
  - evict per (pr,u): DVE reciprocal Z-row (bf16), PE 1-partition bcast
    matmul (ones_bf16[64:65] x rz[64:65]) -> [128,512] psum, DVE mul
    ctx*rz -> ctx_sb f16 [128 = (u,vd), 512] (cross-partition write, u1 ->
    rows 64:128; verified on hw).
  - outproj: f16, 4x128-contraction per [128q, 512od] psum group; DVE f16
    stage -> DMA (psum DMA is forbidden); host sums g-partials + bias.
  - V proj: f16 (value-path precision), 8x128-contraction per [keys, 512].
  - schedule: baseline's filler pacing: proj(s+1) + outproj(s-1//s-2)
    interleaved through attention(s); evict deferred into next pair's QKs.
"""

import os
import sys

try:
    import concourse.bass  # noqa: F401
except ImportError:  # pragma: no cover
    sys.path.insert(0, "/opt/trn_rl_repo")

import numpy as np

B, T, D = 4, 2048, 1024
H, HD = 16, 64
NCORES = 8
NPAIR = 4
NSPAN = 4
SPAN = 512
KC = 128
P = 128
LAG = 3

_CACHE = {}


def _build():
    import concourse.bacc as bacc
    import concourse.mybir as mybir
    import concourse.tile as tile

    f32 = mybir.dt.float32
    f16 = mybir.dt.float16
    bf16 = mybir.dt.bfloat16
    fp8 = mybir.dt.float8e4
    u8 = mybir.dt.uint8
    Exp = mybir.ActivationFunctionType.Exp

    DR = mybir.MatmulPerfMode.DoubleRow

    nc = bacc.Bacc("TRN2", target_bir_lowering=False, debug=False,
                   num_devices=NCORES)

    x8_h = nc.dram_tensor("x8", (D, T), u8, kind="ExternalInput")
    xh_h = nc.dram_tensor("xh", (D, T), f16, kind="ExternalInput")
    wq8_h = nc.dram_tensor("wq8", (D, 512), u8, kind="ExternalInput")
    wk8_h = nc.dram_tensor("wk8", (D, 512), u8, kind="ExternalInput")
    wvh_h = nc.dram_tensor("wvh", (D, 512), f16, kind="ExternalInput")
    woh_h = nc.dram_tensor("woh", (512, D), f16, kind="ExternalInput")
    out_h = nc.dram_tensor("out", (T, D), f16, kind="ExternalOutput")

    # x8: D = (dcp 4, k 2, p 128); per span slice on T
    x8_d = x8_h.ap().rearrange("(dcp k p) t -> p dcp k t", p=P, k=2)
    xh_d = xh_h.ap().rearrange("(dc p) t -> p dc t", p=P)
    wq8_d = wq8_h.ap().rearrange("(dcp k p) (t h m) -> p dcp k t h m",
                                 p=P, k=2, t=2, h=2)
    wk8_d = wk8_h.ap().rearrange("(dcp k p) (t h m) -> p dcp k t h m",
                                 p=P, k=2, t=2, h=2)
    wvh_d = wvh_h.ap().rearrange("(dc p) f -> p dc f", p=P)
    woh_d = woh_h.ap().rearrange("(pc p) f -> p pc f", p=P)

    with tile.TileContext(nc) as tc:
        with (
            tc.tile_pool(name="persist", bufs=1) as persist,
            tc.tile_pool(name="x8p", bufs=3) as x8pool,
            tc.tile_pool(name="xhp", bufs=3) as xhpool,
            tc.tile_pool(name="qp", bufs=6) as qpool,
            tc.tile_pool(name="ptp", bufs=8) as ptpool,
            tc.tile_pool(name="ptb", bufs=5) as ptbpool,
            tc.tile_pool(name="rzp", bufs=8) as rzpool,
            tc.tile_pool(name="cp", bufs=8) as cpool,
            tc.tile_pool(name="stg", bufs=6) as stgpool,
            tc.tile_pool(name="psS", bufs=2, space="PSUM") as psS,
            tc.tile_pool(name="psC", bufs=2, space="PSUM") as psC,
            tc.tile_pool(name="psT", bufs=2, space="PSUM") as psT,
        ):
            wq8 = persist.tile([P, 4, 2, 2, 2, P], u8, tag="wq8", name="wq8")
            wk8 = persist.tile([P, 4, 2, 2, 2, P], u8, tag="wk8", name="wk8")
            wvt = persist.tile([P, 8, 512], f16, tag="wvt", name="wvt")
            wot = persist.tile([P, 4, D], f16, tag="wot", name="wot")
            kT8 = [persist.tile([P, 2, T], fp8, tag=f"kT8{t}", name=f"kT8{t}")
                   for t in range(2)]
            # Vb: [p, kjp 8, kj 2, pr 4, u 2, 96]: 64 v-dims + ones col at
            # 64 + zero pad to 96 (dual-fp8 ldweights needs cols % 32 == 0,
            # >= 64; psum rows 65:96 are dead)
            VW = 96
            Vbh = persist.tile([P, 8, 2, NPAIR, 2, VW], fp8, tag="Vbh",
                               name="Vbh")
            # diag-chunk AV runs in bf16 (pt quantization error bites only
            # concentrated near-diagonal attention rows): [p, kc, pr, u, 65]
            Vbb = persist.tile([P, 16, NPAIR, 2, HD + 1], bf16, tag="Vbb",
                               name="Vbb")
            mask01 = persist.tile([P, KC], bf16, tag="mask01", name="mask01")
            one = nc.const_aps.tensor(1.0, (P, 1))

            # ---- initial DMAs ----
            x8ts = {0: x8pool.tile([P, 4, 2, SPAN], u8, tag="x8t", name="x8t0")}
            xhts = {0: xhpool.tile([P, 8, SPAN], f16, tag="xht", name="xht0")}
            nc.sync.dma_start(wq8[:], wq8_d)
            nc.scalar.dma_start(x8ts[0][:], x8_d[:, :, :, 0:SPAN])
            nc.sync.dma_start(wk8[:], wk8_d)
            nc.scalar.dma_start(xhts[0][:, 0:4], xh_d[:, 0:4, 0:SPAN])
            nc.sync.dma_start(wvt[:], wvh_d)
            nc.scalar.dma_start(xhts[0][:, 4:8], xh_d[:, 4:8, 0:SPAN])
            nc.sync.dma_start(wot[:], woh_d)


            # mask01[p, f] = 1 if p <= f else 0 (post-exp diag pt mask)
            nc.gpsimd.memset(mask01[:], 1.0)
            nc.gpsimd.affine_select(
                out=mask01[:], in_=mask01[:],
                compare_op=mybir.AluOpType.is_ge, fill=0.0,
                base=0, channel_multiplier=-1, pattern=[[1, KC]],
            )
            # Vbh pad cols [65:96] must be zero (read by every off-diag
            # AV); data cols are always written before first read
            nslots = 8 * 2 * NPAIR * 2
            nc.gpsimd.memset(
                Vbh[:].rearrange("p a b c d e -> p (a b c d) e")
                [:, :, HD + 1:], 0.0)
            nc.vector.tensor_copy(
                Vbh[:].rearrange("p a b c d e -> p (a b c d) e")
                [:, :, HD:HD + 1],
                one.to_broadcast((P, nslots, 1)))
            nc.vector.tensor_copy(
                Vbb[:].rearrange("p a c d e -> p (a c d) e")
                [:, :, HD:HD + 1],
                one.to_broadcast((P, 16 * NPAIR * 2, 1)))

            qts = {}    # (sp, t) -> [P, 2, SPAN] fp8 tile
            ctxs = {}   # (sp, pr) -> [P, SPAN] f16 tile

            # ---------- emission helpers ----------
            def proj_qk(w8, pr_half, h, sp, x8t, scale, isq):
                t = pr_half

                def emit():
                    ps = psT.tile([P, SPAN], f32, tag="tr", name="psqk")
                    for dcp in range(4):
                        nc.tensor.matmul(
                            ps[:], w8[:, dcp, :, t, h, :].bitcast(fp8),
                            x8t[:, dcp, :, :].bitcast(fp8),
                            start=(dcp == 0), stop=(dcp == 3), perf_mode=DR)
                    if isq:
                        if (sp, t) not in qts:
                            qts[(sp, t)] = qpool.tile(
                                [P, 2, SPAN], fp8, tag=f"qT{t}",
                                name=f"qT{t}_{sp}")
                        nc.vector.tensor_scalar_mul(
                            qts[(sp, t)][:, h, :], ps[:], scale)
                    else:
                        nc.vector.tensor_scalar_mul(
                            kT8[t][:, h, sp * SPAN:(sp + 1) * SPAN], ps[:],
                            scale)
                return emit

            def proj_v(sp, tb, xht):
                def emit():
                    ps = psT.tile([P, SPAN], f32, tag="tr", name="psv")
                    for dc in range(8):
                        nc.tensor.matmul(
                            ps[:], xht[:, dc, tb * P:(tb + 1) * P],
                            wvt[:, dc, :],
                            start=(dc == 0), stop=(dc == 7))
                    kc = sp * 4 + tb
                    psv = ps[:].rearrange("p (pr u v) -> p pr u v",
                                          pr=NPAIR, v=HD)
                    nc.vector.tensor_copy(
                        Vbh[:, kc // 2, kc % 2, :, :, 0:HD], psv)
                    nc.vector.tensor_copy(Vbb[:, kc, :, :, 0:HD], psv)
                return emit

            def qk_groups(sp, x8t):
                gs = []
                for t in range(2):
                    for h in range(2):
                        gs.append(proj_qk(wq8, t, h, sp, x8t, 1.0 / 32, True))
                        gs.append(proj_qk(wk8, t, h, sp, x8t, 1.0 / 16, False))
                return gs

            def v_groups(sp, xht):
                return [proj_v(sp, tb, xht) for tb in range(4)]

            def outproj_group(sp, tb, os_, dma_q):
                def emit():
                    ps = psT.tile([P, SPAN], f32, tag="tr", name="pso")
                    for pc in range(NPAIR):
                        nc.tensor.matmul(
                            ps[:],
                            ctxs[(sp, pc)][:, (tb - sp * 4) * P:
                                           (tb - sp * 4 + 1) * P],
                            wot[:, pc, os_ * SPAN:(os_ + 1) * SPAN],
                            start=(pc == 0), stop=(pc == NPAIR - 1))
                    stage = stgpool.tile([P, SPAN], f16, tag="st", name="stage")
                    nc.vector.tensor_copy(stage[:], ps[:])
                    dma_q.dma_start(
                        out_h.ap()[tb * P:(tb + 1) * P,
                                   os_ * SPAN:(os_ + 1) * SPAN], stage[:])
                return emit

            def outproj_groups(sp):
                return [outproj_group(sp, tb, os_,
                                      nc.sync if os_ == 0 else nc.gpsimd)
                        for tb in range(sp * 4, (sp + 1) * 4)
                        for os_ in range(2)]

            # ---------- attention for one span ----------
            deferred = []

            def attn_span(s, fillers, vgs=()):
                K = 4 * (s + 1)
                nslot = (K + 4) * NPAIR
                state = {"slot": 0, "fi": 0}

                def pace():
                    tgt = min(len(fillers),
                              len(fillers) * (state["slot"] + 1) // nslot)
                    while state["fi"] < tgt:
                        fillers[state["fi"]]()
                        state["fi"] += 1

                def tick():
                    state["slot"] += 1
                    pace()

                for pr in range(NPAIR):
                    t = pr // 2
                    ctxp = [psC.tile([96, SPAN], f32, tag="ctx",
                                     name=f"ctx{u}") for u in range(2)]
                    ct = cpool.tile([P, SPAN], f16, tag=f"cT{pr}",
                                    name=f"cT{pr}_{s}")
                    ctxs[(s, pr)] = ct
                    pts = {}
                    qt = qts[(s, t)]

                    avn = [0, 0]  # AV matmuls emitted per u (K total each)

                    def emit_qk(kj, pr=pr, t=t, pts=pts, qt=qt):
                        m = kj - 4 * s
                        ss = psS.tile([P, 2, SPAN], f32, tag="psS", name="ss")
                        c0 = 0 if m < 0 else m * KC
                        for u in range(2):
                            b32 = 32 * (2 * (pr % 2) + u)
                            r = slice(b32, b32 + 32)
                            nc.tensor.matmul(
                                ss[:, u, c0:],
                                kT8[t][r, :, kj * KC:(kj + 1) * KC],
                                qt[r, :, c0:],
                                start=True, stop=True, perf_mode=DR,
                                tile_position=(b32, 0))
                        if m < 0:
                            # off-diagonal: fp8 pt pair for DR AV
                            kjp, sl = kj // 2, kj % 2
                            if sl == 0:
                                pts[kjp] = ptpool.tile([P, 2, 2, SPAN], fp8,
                                                       tag="pt", name="pt")
                            nc.scalar.activation(pts[kjp][:, sl, :, :],
                                                 ss[:, :, :], Exp, scale=0.25)
                        else:
                            # diagonal: bf16 pt (exact-ish attention weights
                            # for concentrated rows) + post-exp triangle mask
                            ptb = ptbpool.tile([P, 2, SPAN], bf16,
                                               tag="ptb", name="ptb")
                            pts[("d", kj)] = ptb
                            nc.scalar.activation(ptb[:, :, c0:],
                                                 ss[:, :, c0:], Exp,
                                                 scale=0.25)
                            nc.vector.tensor_mul(
                                ptb[:, :, c0:c0 + KC], ptb[:, :, c0:c0 + KC],
                                mask01[:].rearrange("p (u f) -> p u f", u=1)
                                .to_broadcast((P, 2, KC)))

                    NU = 2 * s + 4  # AV matmuls per u-chain

                    def emit_av8(kjp, pr=pr, pts=pts, ctxp=ctxp, NU=NU):
                        pt = pts.pop(kjp)
                        for u in range(2):
                            nc.tensor.matmul(
                                ctxp[u][:],
                                Vbh[:, kjp, :, pr, u, :],
                                pt[:, :, u, :],
                                start=(avn[u] == 0),
                                stop=(avn[u] == NU - 1),
                                perf_mode=DR, skip_group_check=True)
                            avn[u] += 1

                    def emit_avd(kj, pr=pr, pts=pts, ctxp=ctxp, NU=NU):
                        ptb = pts.pop(("d", kj))
                        c0 = (kj - 4 * s) * KC
                        for u in range(2):
                            nc.tensor.matmul(
                                ctxp[u][0:HD + 1, c0:],
                                Vbb[:, kj, pr, u, :],
                                ptb[:, u, c0:],
                                start=(avn[u] == 0), stop=(avn[u] == NU - 1),
                                skip_group_check=True)
                            avn[u] += 1

                    def evict(pr=pr, ctxp=ctxp, ct=ct, s=s):
                        rz = rzpool.tile([P, 2, SPAN], bf16, tag="rz",
                                         name="rz")
                        rzbc = rzpool.tile([P, 2, SPAN], bf16, tag="rz",
                                           name="rzbc")

                        def fin(u):
                            def run():
                                # 1/Z row: psum row 64 -> sbuf row 0
                                # (cross-partition DVE write, hw-verified)
                                with nc.allow_low_precision(
                                        reason="1/Z bf16: 0.4% on ctx"):
                                    nc.vector.reciprocal(
                                        rz[0:1, u, :], ctxp[u][64:65, :])
                                nc.gpsimd.partition_broadcast(
                                    rzbc[:, u, :], rz[0:1, u, :])
                                nc.vector.tensor_mul(
                                    ct[u * HD:(u + 1) * HD, :],
                                    ctxp[u][0:HD, :], rzbc[0:HD, u, :])
                            return run
                        deferred.append(fin(0))
                        deferred.append(fin(1))

                    # AV work units: (ready_kj, emit_fn); off-diag kjp ready
                    # at its odd kj, diag kj ready at kj
                    units = []
                    for kj2 in range(K):
                        if kj2 < 4 * s:
                            if kj2 % 2 == 1:
                                units.append((kj2, kj2 // 2, emit_av8))
                        else:
                            units.append((kj2, kj2, emit_avd))
                    ui = [0]
                    for kj in range(K):
                        if pr == 0 and kj < len(vgs):
                            # span s's V tiles feed its own diag AV, LAG
                            # slots later -- just-in-time, not paced
                            vgs[kj]()
                        emit_qk(kj)
                        if deferred:
                            deferred.pop(0)()
                        tick()
                        while (ui[0] < len(units)
                               and units[ui[0]][0] + LAG <= kj):
                            units[ui[0]][2](units[ui[0]][1])
                            ui[0] += 1
                    while ui[0] < len(units):
                        if deferred:
                            deferred.pop(0)()
                        tick()
                        units[ui[0]][2](units[ui[0]][1])
                        ui[0] += 1
                    evict()
                while state["fi"] < len(fillers):
                    fillers[state["fi"]]()
                    state["fi"] += 1

            # ---------- main schedule ----------
            # V(s) runs inside span s itself (its first consumer is span s's
            # own diag AV, ~LAG kjs in) -- keeps spans 0/1 off the PE
            for g in qk_groups(0, x8ts[0]):
                g()
            for s in range(NSPAN):
                vgs = v_groups(s, xhts[s])
                fillers = []
                if s + 1 < NSPAN:
                    x8t = x8pool.tile([P, 4, 2, SPAN], u8, tag="x8t",
                                      name=f"x8t{s + 1}")
                    xht = xhpool.tile([P, 8, SPAN], f16, tag="xht",
                                      name=f"xht{s + 1}")
                    x8ts[s + 1], xhts[s + 1] = x8t, xht
                    sl = slice((s + 1) * SPAN, (s + 2) * SPAN)
                    nc.sync.dma_start(x8t[:], x8_d[:, :, :, sl])
                    nc.scalar.dma_start(xht[:, 0:4], xh_d[:, 0:4, sl])
                    nc.scalar.dma_start(xht[:, 4:8], xh_d[:, 4:8, sl])
                    fillers += qk_groups(s + 1, x8t)
                if s == 2:
                    fillers += outproj_groups(0)
                elif s == 3:
                    fillers += outproj_groups(1) + outproj_groups(2)
                attn_span(s, fillers, vgs)
            while deferred:
                deferred.pop(0)()
            for g in outproj_groups(3):
                g()

    nc.compile()
    return nc


def get_nc():
    if "nc" not in _CACHE:
        _CACHE["nc"] = _build()
    return _CACHE["nc"]


def _perm512():
    perm = np.empty(512, np.int64)
    i = 0
    for t in range(2):
        for h in range(2):
            for beta in range(4):
                pr = 2 * t + beta // 2
                u = beta % 2
                for dd in range(32):
                    perm[i] = pr * 128 + u * 64 + h * 32 + dd
                    i += 1
    return perm


def kernel(x, Wq, Wk, Wv, Wo, bo):
    import ml_dtypes
    from concourse import bass_utils

    e4 = ml_dtypes.float8_e4m3

    x = np.asarray(x, dtype=np.float32)
    Wq, Wk, Wv = (np.asarray(w, dtype=np.float32) for w in (Wq, Wk, Wv))
    Wo = np.asarray(Wo, dtype=np.float32)
    bo = np.asarray(bo, dtype=np.float32)
    perm = _perm512()

    in_maps = []
    for c in range(NCORES):
        b, g = c // 2, c % 2
        gsl = slice(g * 512, (g + 1) * 512)
        xT = np.ascontiguousarray(x[b].T)
        in_maps.append({
            "x8": xT.astype(e4).view(np.uint8),
            "xh": xT.astype(np.float16),
            # w quantized x16 to dodge e4m3 subnormals; unscaled on-chip
            "wq8": np.ascontiguousarray((16.0 * Wq[gsl].T)[:, perm])
            .astype(e4).view(np.uint8),
            "wk8": np.ascontiguousarray((16.0 * Wk[gsl].T)[:, perm])
            .astype(e4).view(np.uint8),
            "wvh": np.ascontiguousarray(Wv[gsl].T).astype(np.float16),
            "woh": np.ascontiguousarray(Wo[:, gsl].T).astype(np.float16),
        })

    nc = get_nc()
    res = bass_utils.run_bass_kernel_spmd(nc, in_maps,
                                          core_ids=list(range(NCORES)))
    parts = [res.results[c]["out"].astype(np.float32) for c in range(NCORES)]
    out = np.stack([parts[2 * b] + parts[2 * b + 1] + bo for b in range(B)])
    return out.astype(np.float32)


# revision 29
# speedup vs baseline: 1.0428x; 1.0428x over previous
"""Multi-head causal attention (B=4, T=2048, D=1024, H=16) on 8 Trainium2 cores.

Sharding: core c = (b, g) with b = c//2 (batch), g = c%2 (head-group of 8 heads).
Each core: Q/K/V projections for its 8 heads (column-parallel), causal attention,
row-parallel partial output projection. Host sums the g=0/g=1 partials + bias.

v4 design (fp8 DoubleRow + streaming AV; cost model: matmul = out-free-rows x
0.4167ns x cpr, fp8e4 DoubleRow cpr=0.5 contracting 2x128/instr; ACT exp =
0.8333ns/free-elem is the ~146us floor):
  - Q/K proj: fp8 DR, x8 moving [128,2,512], w8 stationary [128,2,128] in
    4 col-groups (t=pair-half, h=dim-half) so psum partitions land as
    (beta=2*(pr%2)+u, dd) blocks of 32 -> qT8/kT8 [32-blocks, 2h, T] fp8,
    ready for 2x32-contraction DR score matmuls. w quantized x16 (subnormal
    dodge), unscaled in the DVE psum->fp8 copy; score scale 1/8 folded as
    qT8 = e4m3(q/2) + exp(scale=0.25).
  - scores: fp8 DR per (pr,u,kj): lhsT = kT8 32x2x128, rhs = qT8, out ss
    [128 keys, u, 512-c0] psum; diag mask = DVE add of -1e9 maskb pre-exp.
  - exp: ACT psum->sbuf fp8 pt pairs [128, 2kj, 2u, 512]; diag garbage cols
    zeroed by Pool memset (bitwise, fp8-safe).
  - AV: streaming fp8 DR per (pr,u,kjp,hi/lo): lhsT = Vb [128,2kj,65]
    (64 v-dims + ones col -> Z at psum row 64), rhs = pt pair, out ctx
    [65, 512] psum accumulated across the span. V stored hi+lo e4m3 pair
    (residual split ~ fp16 quality at DR cost).
  - evict per (pr,u): DVE reciprocal Z-row (bf16), PE 1-partition bcast
    matmul (ones_bf16[64:65] x rz[64:65]) -> [128,512] psum, DVE mul
    ctx*rz -> ctx_sb f16 [128 = (u,vd), 512] (cross-partition write, u1 ->
    rows 64:128; verified on hw).
  - outproj: f16, 4x128-contraction per [128q, 512od] psum group; DVE f16
    stage -> DMA (psum DMA is forbidden); host sums g-partials + bias.
  - V proj: f16 (value-path precision), 8x128-contraction per [keys, 512].
  - schedule: baseline's filler pacing: proj(s+1) + outproj(s-1//s-2)
    interleaved through attention(s); evict deferred into next pair's QKs.
"""

import os
import sys

try:
    import concourse.bass  # noqa: F401
except ImportError:  # pragma: no cover
    sys.path.insert(0, "/opt/trn_rl_repo")

import numpy as np

B, T, D = 4, 2048, 1024
H, HD = 16, 64
NCORES = 8
NPAIR = 4
NSPAN = 4
SPAN = 512
KC = 128
P = 128
LAG = 3

_CACHE = {}


def _build():
    import concourse.bacc as bacc
    import concourse.mybir as mybir
    import concourse.tile as tile

    f32 = mybir.dt.float32
    f16 = mybir.dt.float16
    bf16 = mybir.dt.bfloat16
    fp8 = mybir.dt.float8e4
    u8 = mybir.dt.uint8
    Exp = mybir.ActivationFunctionType.Exp

    DR = mybir.MatmulPerfMode.DoubleRow

    nc = bacc.Bacc("TRN2", target_bir_lowering=False, debug=False,
                   num_devices=NCORES)

    x8_h = nc.dram_tensor("x8", (D, T), u8, kind="ExternalInput")
    xh_h = nc.dram_tensor("xh", (D, T), f16, kind="ExternalInput")
    wq8_h = nc.dram_tensor("wq8", (D, 512), u8, kind="ExternalInput")
    wk8_h = nc.dram_tensor("wk8", (D, 512), u8, kind="ExternalInput")
    wvh_h = nc.dram_tensor("wvh", (D, 512), f16, kind="ExternalInput")
    woh_h = nc.dram_tensor("woh", (512, D), f16, kind="ExternalInput")
    out_h = nc.dram_tensor("out", (T, D), f16, kind="ExternalOutput")

    # x8: D = (dcp 4, k 2, p 128); per span slice on T
    x8_d = x8_h.ap().rearrange("(dcp k p) t -> p dcp k t", p=P, k=2)
    xh_d = xh_h.ap().rearrange("(dc p) t -> p dc t", p=P)
    wq8_d = wq8_h.ap().rearrange("(dcp k p) (t h m) -> p dcp k t h m",
                                 p=P, k=2, t=2, h=2)
    wk8_d = wk8_h.ap().rearrange("(dcp k p) (t h m) -> p dcp k t h m",
                                 p=P, k=2, t=2, h=2)
    wvh_d = wvh_h.ap().rearrange("(dc p) f -> p dc f", p=P)
    woh_d = woh_h.ap().rearrange("(pc p) f -> p pc f", p=P)

    with tile.TileContext(nc) as tc:
        with (
            tc.tile_pool(name="persist", bufs=1) as persist,
            tc.tile_pool(name="x8p", bufs=2) as x8pool,
            tc.tile_pool(name="xhp", bufs=2) as xhpool,
            tc.tile_pool(name="qp", bufs=4) as qpool,
            tc.tile_pool(name="ptp", bufs=6) as ptpool,
            tc.tile_pool(name="ptb", bufs=5) as ptbpool,
            tc.tile_pool(name="rzp", bufs=8) as rzpool,
            tc.tile_pool(name="cp", bufs=8) as cpool,
            tc.tile_pool(name="stg", bufs=6) as stgpool,
            tc.tile_pool(name="psS", bufs=2, space="PSUM") as psS,
            tc.tile_pool(name="psC", bufs=2, space="PSUM") as psC,
            tc.tile_pool(name="psT", bufs=2, space="PSUM") as psT,
        ):
            wq8 = persist.tile([P, 4, 2, 2, 2, P], u8, tag="wq8", name="wq8")
            wk8 = persist.tile([P, 4, 2, 2, 2, P], u8, tag="wk8", name="wk8")
            wvt = persist.tile([P, 8, 512], f16, tag="wvt", name="wvt")
            wot = persist.tile([P, 4, D], f16, tag="wot", name="wot")
            kT8 = [persist.tile([P, 2, T], fp8, tag=f"kT8{t}", name=f"kT8{t}")
                   for t in range(2)]
            # Vb: [p, kjp 8, kj 2, pr 4, u 2, 96]: 64 v-dims + ones col at
            # 64 + zero pad to 96 (dual-fp8 ldweights needs cols % 32 == 0,
            # >= 64; psum rows 65:96 are dead)
            VW = 96
            Vbh = persist.tile([P, 8, 2, NPAIR, 2, VW], fp8, tag="Vbh",
                               name="Vbh")
            # diag-chunk AV runs in bf16 (pt quantization error bites only
            # concentrated near-diagonal attention rows): [p, kc, pr, u, 65]
            Vbb = persist.tile([P, 16, NPAIR, 2, HD + 1], bf16, tag="Vbb",
                               name="Vbb")
            mask01 = persist.tile([P, KC], bf16, tag="mask01", name="mask01")
            one = nc.const_aps.tensor(1.0, (P, 1))

            # ---- initial DMAs ----
            x8ts = {0: x8pool.tile([P, 4, 2, SPAN], u8, tag="x8t", name="x8t0")}
            xhts = {0: xhpool.tile([P, 8, SPAN], f16, tag="xht", name="xht0")}
            nc.sync.dma_start(wq8[:], wq8_d)
            nc.scalar.dma_start(x8ts[0][:], x8_d[:, :, :, 0:SPAN])
            nc.sync.dma_start(wk8[:], wk8_d)
            nc.scalar.dma_start(xhts[0][:, 0:4], xh_d[:, 0:4, 0:SPAN])
            nc.sync.dma_start(wvt[:], wvh_d)
            nc.scalar.dma_start(xhts[0][:, 4:8], xh_d[:, 4:8, 0:SPAN])
            nc.sync.dma_start(wot[:], woh_d)


            # mask01[p, f] = 1 if p <= f else 0 (post-exp diag pt mask)
            nc.gpsimd.memset(mask01[:], 1.0)
            nc.gpsimd.affine_select(
                out=mask01[:], in_=mask01[:],
                compare_op=mybir.AluOpType.is_ge, fill=0.0,
                base=0, channel_multiplier=-1, pattern=[[1, KC]],
            )
            # Vbh pad cols [65:96] must be zero (read by every off-diag
            # AV); data cols are always written before first read
            nslots = 8 * 2 * NPAIR * 2
            nc.gpsimd.memset(
                Vbh[:].rearrange("p a b c d e -> p (a b c d) e")
                [:, :, HD + 1:], 0.0)
            nc.vector.tensor_copy(
                Vbh[:].rearrange("p a b c d e -> p (a b c d) e")
                [:, :, HD:HD + 1],
                one.to_broadcast((P, nslots, 1)))
            nc.vector.tensor_copy(
                Vbb[:].rearrange("p a c d e -> p (a c d) e")
                [:, :, HD:HD + 1],
                one.to_broadcast((P, 16 * NPAIR * 2, 1)))

            qts = {}    # (sp, t) -> [P, 2, SPAN] fp8 tile
            ctxs = {}   # (sp, pr) -> [P, SPAN] f16 tile

            # ---------- emission helpers ----------
            def proj_qk(w8, pr_half, h, sp, x8t, scale, isq):
                t = pr_half

                def emit():
                    ps = psT.tile([P, SPAN], f32, tag="tr", name="psqk")
                    for dcp in range(4):
                        nc.tensor.matmul(
                            ps[:], w8[:, dcp, :, t, h, :].bitcast(fp8),
                            x8t[:, dcp, :, :].bitcast(fp8),
                            start=(dcp == 0), stop=(dcp == 3), perf_mode=DR)
                    if isq:
                        if (sp, t) not in qts:
                            qts[(sp, t)] = qpool.tile(
                                [P, 2, SPAN], fp8, tag=f"qT{t}",
                                name=f"qT{t}_{sp}")
                        nc.vector.tensor_scalar_mul(
                            qts[(sp, t)][:, h, :], ps[:], scale)
                    else:
                        nc.vector.tensor_scalar_mul(
                            kT8[t][:, h, sp * SPAN:(sp + 1) * SPAN], ps[:],
                            scale)
                return emit

            def proj_v(sp, tb, xht):
                def emit():
                    ps = psT.tile([P, SPAN], f32, tag="tr", name="psv")
                    for dc in range(8):
                        nc.tensor.matmul(
                            ps[:], xht[:, dc, tb * P:(tb + 1) * P],
                            wvt[:, dc, :],
                            start=(dc == 0), stop=(dc == 7))
                    kc = sp * 4 + tb
                    psv = ps[:].rearrange("p (pr u v) -> p pr u v",
                                          pr=NPAIR, v=HD)
                    nc.vector.tensor_copy(
                        Vbh[:, kc // 2, kc % 2, :, :, 0:HD], psv)
                    nc.vector.tensor_copy(Vbb[:, kc, :, :, 0:HD], psv)
                return emit

            def qk_groups(sp, x8t):
                gs = []
                for t in range(2):
                    for h in range(2):
                        gs.append(proj_qk(wq8, t, h, sp, x8t, 1.0 / 32, True))
                        gs.append(proj_qk(wk8, t, h, sp, x8t, 1.0 / 16, False))
                return gs

            def v_groups(sp, xht):
                return [proj_v(sp, tb, xht) for tb in range(4)]

            def outproj_group(sp, tb, os_, dma_q):
                def emit():
                    ps = psT.tile([P, SPAN], f32, tag="tr", name="pso")
                    for pc in range(NPAIR):
                        nc.tensor.matmul(
                            ps[:],
                            ctxs[(sp, pc)][:, (tb - sp * 4) * P:
                                           (tb - sp * 4 + 1) * P],
                            wot[:, pc, os_ * SPAN:(os_ + 1) * SPAN],
                            start=(pc == 0), stop=(pc == NPAIR - 1))
                    stage = stgpool.tile([P, SPAN], f16, tag="st", name="stage")
                    nc.vector.tensor_copy(stage[:], ps[:])
                    dma_q.dma_start(
                        out_h.ap()[tb * P:(tb + 1) * P,
                                   os_ * SPAN:(os_ + 1) * SPAN], stage[:])
                return emit

            def outproj_groups(sp):
                return [outproj_group(sp, tb, os_,
                                      nc.sync if os_ == 0 else nc.gpsimd)
                        for tb in range(sp * 4, (sp + 1) * 4)
                        for os_ in range(2)]

            # ---------- attention for one span ----------
            deferred = []

            def attn_span(s, fillers, vgs=()):
                K = 4 * (s + 1)
                nslot = (K + 4) * NPAIR
                state = {"slot": 0, "fi": 0}

                def pace():
                    tgt = min(len(fillers),
                              len(fillers) * (state["slot"] + 1) // nslot)
                    while state["fi"] < tgt:
                        fillers[state["fi"]]()
                        state["fi"] += 1

                def tick():
                    state["slot"] += 1
                    pace()

                for pr in range(NPAIR):
                    t = pr // 2
                    ctxp = [psC.tile([96, SPAN], f32, tag="ctx",
                                     name=f"ctx{u}") for u in range(2)]
                    ct = cpool.tile([P, SPAN], f16, tag=f"cT{pr}",
                                    name=f"cT{pr}_{s}")
                    ctxs[(s, pr)] = ct
                    pts = {}
                    qt = qts[(s, t)]

                    avn = [0, 0]  # AV matmuls emitted per u (K total each)

                    def emit_qk(kj, pr=pr, t=t, pts=pts, qt=qt):
                        m = kj - 4 * s
                        ss = psS.tile([P, 2, SPAN], f32, tag="psS", name="ss")
                        c0 = 0 if m < 0 else m * KC
                        for u in range(2):
                            b32 = 32 * (2 * (pr % 2) + u)
                            r = slice(b32, b32 + 32)
                            nc.tensor.matmul(
                                ss[:, u, c0:],
                                kT8[t][r, :, kj * KC:(kj + 1) * KC],
                                qt[r, :, c0:],
                                start=True, stop=True, perf_mode=DR,
                                tile_position=(b32, 0))
                        if m < 0:
                            # off-diagonal: fp8 pt pair for DR AV
                            kjp, sl = kj // 2, kj % 2
                            if sl == 0:
                                pts[kjp] = ptpool.tile([P, 2, 2, SPAN], fp8,
                                                       tag="pt", name="pt")
                            nc.scalar.activation(pts[kjp][:, sl, :, :],
                                                 ss[:, :, :], Exp, scale=0.25)
                        else:
                            # diagonal: bf16 pt (exact-ish attention weights
                            # for concentrated rows) + post-exp triangle mask
                            ptb = ptbpool.tile([P, 2, SPAN], bf16,
                                               tag="ptb", name="ptb")
                            pts[("d", kj)] = ptb
                            nc.scalar.activation(ptb[:, :, c0:],
                                                 ss[:, :, c0:], Exp,
                                                 scale=0.25)
                            nc.vector.tensor_mul(
                                ptb[:, :, c0:c0 + KC], ptb[:, :, c0:c0 + KC],
                                mask01[:].rearrange("p (u f) -> p u f", u=1)
                                .to_broadcast((P, 2, KC)))

                    NU = 2 * s + 4  # AV matmuls per u-chain

                    def emit_av8(kjp, pr=pr, pts=pts, ctxp=ctxp, NU=NU):
                        pt = pts.pop(kjp)
                        for u in range(2):
                            nc.tensor.matmul(
                                ctxp[u][:],
                                Vbh[:, kjp, :, pr, u, :],
                                pt[:, :, u, :],
                                start=(avn[u] == 0),
                                stop=(avn[u] == NU - 1),
                                perf_mode=DR, skip_group_check=True)
                            avn[u] += 1

                    def emit_avd(kj, pr=pr, pts=pts, ctxp=ctxp, NU=NU):
                        ptb = pts.pop(("d", kj))
                        c0 = (kj - 4 * s) * KC
                        for u in range(2):
                            nc.tensor.matmul(
                                ctxp[u][0:HD + 1, c0:],
                                Vbb[:, kj, pr, u, :],
                                ptb[:, u, c0:],
                                start=(avn[u] == 0), stop=(avn[u] == NU - 1),
                                skip_group_check=True)
                            avn[u] += 1

                    def evict(pr=pr, ctxp=ctxp, ct=ct, s=s):
                        rz = rzpool.tile([P, 2, SPAN], bf16, tag="rz",
                                         name="rz")
                        rzbc = rzpool.tile([P, 2, SPAN], bf16, tag="rz",
                                           name="rzbc")

                        def fin(u):
                            def run():
                                # 1/Z row: psum row 64 -> sbuf row 0
                                # (cross-partition DVE write, hw-verified)
                                with nc.allow_low_precision(
                                        reason="1/Z bf16: 0.4% on ctx"):
                                    nc.vector.reciprocal(
                                        rz[0:1, u, :], ctxp[u][64:65, :])
                                nc.gpsimd.partition_broadcast(
                                    rzbc[:, u, :], rz[0:1, u, :])
                                nc.vector.tensor_mul(
                                    ct[u * HD:(u + 1) * HD, :],
                                    ctxp[u][0:HD, :], rzbc[0:HD, u, :])
                            return run
                        deferred.append(fin(0))
                        deferred.append(fin(1))

                    # AV work units: (ready_kj, emit_fn); off-diag kjp ready
                    # at its odd kj, diag kj ready at kj
                    units = []
                    for kj2 in range(K):
                        if kj2 < 4 * s:
                            if kj2 % 2 == 1:
                                units.append((kj2, kj2 // 2, emit_av8))
                        else:
                            units.append((kj2, kj2, emit_avd))
                    ui = [0]
                    for kj in range(K):
                        if pr == 0 and kj < len(vgs):
                            # span s's V tiles feed its own diag AV, LAG
                            # slots later -- just-in-time, not paced
                            vgs[kj]()
                        emit_qk(kj)
                        if deferred:
                            deferred.pop(0)()
                        tick()
                        while (ui[0] < len(units)
                               and units[ui[0]][0] + LAG <= kj):
                            units[ui[0]][2](units[ui[0]][1])
                            ui[0] += 1
                    while ui[0] < len(units):
                        if deferred:
                            deferred.pop(0)()
                        tick()
                        units[ui[0]][2](units[ui[0]][1])
                        ui[0] += 1
                    evict()
                while state["fi"] < len(fillers):
                    fillers[state["fi"]]()
                    state["fi"] += 1

            # ---------- main schedule ----------
            # V(s) runs inside span s itself (its first consumer is span s's
            # own diag AV, ~LAG kjs in) -- keeps spans 0/1 off the PE
            for g in qk_groups(0, x8ts[0]):
                g()
            for s in range(NSPAN):
                vgs = v_groups(s, xhts[s])
                fillers = []
                if s + 1 < NSPAN:
                    x8t = x8pool.tile([P, 4, 2, SPAN], u8, tag="x8t",
                                      name=f"x8t{s + 1}")
                    xht = xhpool.tile([P, 8, SPAN], f16, tag="xht",
                                      name=f"xht{s + 1}")
                    x8ts[s + 1], xhts[s + 1] = x8t, xht
                    sl = slice((s + 1) * SPAN, (s + 2) * SPAN)
                    nc.sync.dma_start(x8t[:], x8_d[:, :, :, sl])
                    nc.scalar.dma_start(xht[:, 0:4], xh_d[:, 0:4, sl])
                    nc.scalar.dma_start(xht[:, 4:8], xh_d[:, 4:8, sl])
                    fillers += qk_groups(s + 1, x8t)
                if s == 2:
                    fillers += outproj_groups(0)
                elif s == 3:
                    fillers += outproj_groups(1) + outproj_groups(2)
                attn_span(s, fillers, vgs)
            while deferred:
                deferred.pop(0)()
            for g in outproj_groups(3):
                g()

    nc.compile()
    return nc


def get_nc():
    if "nc" not in _CACHE:
        _CACHE["nc"] = _build()
    return _CACHE["nc"]


def _perm512():
    perm = np.empty(512, np.int64)
    i = 0
    for t in range(2):
        for h in range(2):
            for beta in range(4):
                pr = 2 * t + beta // 2
                u = beta % 2
                for dd in range(32):
                    perm[i] = pr * 128 + u * 64 + h * 32 + dd
                    i += 1
    return perm


def kernel(x, Wq, Wk, Wv, Wo, bo):
    import ml_dtypes
    from concourse import bass_utils

    e4 = ml_dtypes.float8_e4m3

    x = np.asarray(x, dtype=np.float32)
    Wq, Wk, Wv = (np.asarray(w, dtype=np.float32) for w in (Wq, Wk, Wv))
    Wo = np.asarray(Wo, dtype=np.float32)
    bo = np.asarray(bo, dtype=np.float32)
    perm = _perm512()

    in_maps = []
    for c in range(NCORES):
        b, g = c // 2, c % 2
        gsl = slice(g * 512, (g + 1) * 512)
        xT = np.ascontiguousarray(x[b].T)
        in_maps.append({
            "x8": xT.astype(e4).view(np.uint8),
            "xh": xT.astype(np.float16),
            # w quantized x16 to dodge e4m3 subnormals; unscaled on-chip
            "wq8": np.ascontiguousarray((16.0 * Wq[gsl].T)[:, perm])
            .astype(e4).view(np.uint8),
            "wk8": np.ascontiguousarray((16.0 * Wk[gsl].T)[:, perm])
            .astype(e4).view(np.uint8),
            "wvh": np.ascontiguousarray(Wv[gsl].T).astype(np.float16),
            "woh": np.ascontiguousarray(Wo[:, gsl].T).astype(np.float16),
        })

    nc = get_nc()
    res = bass_utils.run_bass_kernel_spmd(nc, in_maps,
                                          core_ids=list(range(NCORES)))
    parts = [res.results[c]["out"].astype(np.float32) for c in range(NCORES)]
    out = np.stack([parts[2 * b] + parts[2 * b + 1] + bo for b in range(B)])
    return out.astype(np.float32)
